# revision 1
# baseline (speedup 1.0000x reference)
# Trainium2 Bass kernel for nn_DE_Func_25323127177649.
#
# Architecture (B=8192, XD=ZD=32, H=64):
#   - per-dim grouped 2-layer MLPs (encoders / extractors / xdot) with tanh/elu
#   - shared 4-layer "V" MLP contracting across the 3*(XD+ZD) channel axis
#
# Device mapping (pure batch data-parallel over 8 cores, 1024 batch each):
#   - activations live feature-major [feat(part), batch(free)]; group pairs
#     (2j, 2j+1) are stacked on the 128 partitions and processed with
#     block-diagonal [128,128] fp32r weights (one matmul per pair).  This
#     keeps every matmul output at partition 0 (this walrus rejects fp32r
#     matmuls with nonzero column tile positions).
#   - host pre-fuses consecutive linear layers (encoder-L2 @ extractor-L1),
#     folds the cat3 diff into V1 (V1p = V1a+V1c, V1q = V1b-V1c), and
#     rewrites elu as elu'(y) = elu(y)+1 = min(exp(y), 1+relu(y)) with the
#     "-1" folded into the consumer's bias.
#   - the group-major <-> channel-major layout switch around the V-MLP is
#     done with SBUF->SBUF DMAs (partition collapse/expand), h-major column
#     order so each f-tile flattens onto contiguous k-rows.
#   - walrus here encodes at most ONE sync wait per instruction; a post-pass
#     splits Tile's multi-wait instructions into standalone wait-NoOps.
import numpy as np
import ml_dtypes

import concourse.bass as bass
import concourse.mybir as mybir
import concourse.tile as tile
from concourse.bass_utils import run_bass_kernel_spmd

dt = mybir.dt
AF = mybir.ActivationFunctionType
ALU = mybir.AluOpType

B, XD, ZD, H = 8192, 32, 32, 64
NCORES = 8
BC = B // NCORES          # batch per core
NB = 256                  # batch tile (matmul free dim; fp32r needs >=256)
NT = BC // NB             # batch tiles per core
NPAIR = 16                # group pairs (32 groups / 2)
NCHUNK = H                # V-stage chunks per batch tile (h-major: chunk == h)

F32, BF16, F32R = dt.float32, dt.bfloat16, dt.float32r


# ---- packed-constant layout: name -> (pack, col offset, width, rows) ----
def _mk_layout():
    layout = {}
    offs = {"packR": 0, "packF": 0, "packB": 0}

    def add(nm, pk, w, rows=128):
        layout[nm] = (pk, offs[pk], w, rows)
        offs[pk] += w

    add("x0r", "packR", BC)
    add("z0r", "packR", BC)
    add("ztr", "packR", BC)
    add("wx1m", "packR", NPAIR * 128)   # xenc L1 masked [32,128] pair blocks
    add("wz1m", "packR", NPAIR * 128)
    add("wxf", "packR", NPAIR * 128)    # block-diag pair stacks
    add("wzf", "packR", NPAIR * 128)
    add("wxe1", "packR", NPAIR * 128)
    add("wxe2", "packR", NPAIR * 128)
    add("wze2", "packR", NPAIR * 128)
    add("wxd1", "packR", NPAIR * 128)
    add("wxd2", "packR", NPAIR * 128)
    add("v2s", "packR", 128)            # diag(V2,V2)
    add("v3s", "packR", 128)
    add("v4s", "packR", 64)             # diag(V4,V4) -> M=64
    for nm in ("bxt", "bzt", "bfx_e", "bfx_r", "bfz_e", "bfz_r",
               "bx1_e", "bx1_r", "b2x", "b2z", "bd1_e", "bd1_r", "b2d"):
        add(nm, "packF", NPAIR)
    for nm in ("bv1_e", "bv1_r", "bv2_e", "bv2_r", "bv3_e", "bv3_r", "bv4"):
        add(nm, "packF", 1)
    add("v1e", "packB", H)
    return layout, offs["packR"], offs["packF"], offs["packB"]


CONST_LAYOUT, PACKR_W, PACKF_W, PACKB_W = _mk_layout()


def _split_multi_waits(nc):
    """walrus encodes at most one sync-wait per instruction; hoist extras
    onto standalone NoOps on the same engine queue."""
    for fn in nc.m.functions:
        for blk in fn.blocks:
            out = []
            for inst in blk.instructions:
                si = inst.sync_info
                waits = list(si.on_wait) if si and si.on_wait else []
                if len(waits) > 1:
                    for w in waits[:-1]:
                        out.append(mybir.InstNoOp(
                            name=nc.get_next_instruction_name(),
                            engine=inst.engine,
                            sync_info=mybir.SyncInfo(on_wait=[w], on_update=[]),
                            bass_nofuse=True,
                        ))
                    inst.sync_info = mybir.SyncInfo(
                        on_wait=[waits[-1]], on_update=list(si.on_update or []))
                out.append(inst)
            blk.instructions = out


def _build_nc(split_waits=True):
    nc = bass.Bass("TRN2", target_bir_lowering=False, debug=False,
                   enable_asserts=False)
    io = {}

    def inp(name, shape, dtype=F32):
        io[name] = nc.dram_tensor(name, list(shape), dtype,
                                  kind="ExternalInput").ap()
        return io[name]

    inp("xhtT", (XD, H, BC), F32R)      # Xht, group-major [i, h, b]
    inp("packR", (128, PACKR_W), F32R)
    inp("packF", (128, PACKF_W), F32)
    inp("packB", (128, PACKB_W), BF16)

    out = nc.dram_tensor("outT", [XD, H, BC], F32, kind="ExternalOutput").ap()
    io["outT"] = out

    with tile.TileContext(nc) as tc:
        _kernel_body(nc, tc, io)
    if split_waits:
        _split_multi_waits(nc)
    return nc


def _kernel_body(nc, tc, io):
    with (
        tc.tile_pool(name="const", bufs=1) as cpool,
        tc.tile_pool(name="inio", bufs=4) as iopool,
        tc.tile_pool(name="work", bufs=2) as wpool,
        tc.tile_pool(name="fout", bufs=4) as fpool,
        tc.tile_pool(name="big", bufs=1) as bigpool,
        tc.tile_pool(name="ps", bufs=7, space="PSUM") as ppool,
    ):
        packs = {}
        for nm in ("packR", "packF", "packB"):
            ap = io[nm]
            t = cpool.tile(list(ap.shape), ap.dtype, name=f"c_{nm}")
            nc.sync.dma_start(out=t[:], in_=ap[:])
            packs[nm] = t
        C = {}
        for nm, (pk, off, w, rows) in CONST_LAYOUT.items():
            C[nm] = packs[pk][0:rows, off:off + w]

        def ps_tile(nm, shape=(128, 2 * NB)):
            return ppool.tile(list(shape), F32, name=nm, tag="ps")

        def bd_mm(wstk, j, rhs, ps_slice):
            """One block-diag pair matmul: lhsT [128,128], out [128, NB]."""
            nc.tensor.matmul(ps_slice, lhsT=wstk[:, j * 128:(j + 1) * 128],
                             rhs=rhs, start=True, stop=True,
                             tile_position=(0, 0))

        def elu_evict(ps, be, br):
            """elu'(ps + bias) = min(exp(ps+be), max(ps+br, 1)); [128, NB]."""
            E = wpool.tile([128, NB], F32, name="E", tag="E")
            nc.scalar.activation(E[:], ps[:], AF.Exp, bias=be)
            R = wpool.tile([128, NB], F32, name="R", tag="R")
            nc.vector.tensor_scalar(R[:], ps[:], br, 1.0, ALU.add, ALU.max)
            O = wpool.tile([128, NB], F32R, name="O", tag="O")
            nc.vector.tensor_tensor(O[:], E[:], R[:], ALU.min)
            return O

        for t in range(NT):
            tsl = slice(t * NB, (t + 1) * NB)

            rhsV = bigpool.tile([128, NCHUNK * NB], BF16, name="rhsV", tag="rhsV")
            XR = bigpool.tile([128, (XD // 2) * NB], F32R, name="XR", tag="XR")

            # ---------- encoder paths (x0, z0, zt) + Xht path -> f rows ----------
            # k-row bases in rhsV: f_Xht 0, f_Zht 32, f_Xh0 64, f_Zh0 96
            paths = (
                ("x0", "x0r", "wx1m", "bxt", "wxf", "bfx_e", "bfx_r",
                 "wxe2", "b2x", 64),
                ("z0", "z0r", "wz1m", "bzt", "wzf", "bfz_e", "bfz_r",
                 "wze2", "b2z", 96),
                ("zt", "ztr", "wz1m", "bzt", "wzf", "bfz_e", "bfz_r",
                 "wze2", "b2z", 32),
            )
            for (pname, zrep_n, w1m_n, bt_n, wf_n, bfe_n, bfr_n,
                 w2_n, b2_n, kbase) in paths:
                zrep, w1m, bt = C[zrep_n], C[w1m_n], C[bt_n]
                wf, bfe, bfr = C[wf_n], C[bfe_n], C[bfr_n]
                w2, b2 = C[w2_n], C[b2_n]
                for j in range(NPAIR):
                    s = j % 4
                    psA = ps_tile("psA", (128, NB))
                    nc.tensor.matmul(
                        psA[:],
                        lhsT=w1m[32 * s:32 * s + 32, j * 128:(j + 1) * 128],
                        rhs=zrep[32 * s:32 * s + 32, tsl],
                        start=True, stop=True, tile_position=(32 * s, 0))
                    A = wpool.tile([128, NB], F32R, name="A", tag="A")
                    nc.scalar.activation(A[:], psA[:], AF.Tanh,
                                         bias=bt[:, j:j + 1])
                    psB = ps_tile("psB", (128, NB))
                    bd_mm(wf, j, A[:], psB[:])
                    Ee = elu_evict(psB, bfe[:, j:j + 1], bfr[:, j:j + 1])
                    psC = ps_tile("psC", (128, NB))
                    bd_mm(w2, j, Ee[:], psC[:])
                    fT = fpool.tile([128, NB], F32, name="fT", tag="fT")
                    nc.scalar.activation(fT[:], psC[:], AF.Identity,
                                         bias=b2[:, j:j + 1])
                    k0 = kbase + 2 * j
                    nc.gpsimd.dma_start(out=rhsV[k0:k0 + 2, :], in_=fT[:])

            for j in range(NPAIR):  # Xht path
                xa = iopool.tile([128, NB], F32R, name="xa", tag="xa")
                nc.sync.dma_start(out=xa[0:64, :], in_=io["xhtT"][2 * j, :, tsl])
                nc.sync.dma_start(out=xa[64:128, :],
                                  in_=io["xhtT"][2 * j + 1, :, tsl])
                psD = ps_tile("psD", (128, NB))
                bd_mm(C["wxe1"], j, xa[:], psD[:])
                Ex = elu_evict(psD, C["bx1_e"][:, j:j + 1], C["bx1_r"][:, j:j + 1])
                psE = ps_tile("psE", (128, NB))
                bd_mm(C["wxe2"], j, Ex[:], psE[:])
                fT = fpool.tile([128, NB], F32, name="fT", tag="fT")
                nc.scalar.activation(fT[:], psE[:], AF.Identity,
                                     bias=C["b2x"][:, j:j + 1])
                nc.gpsimd.dma_start(out=rhsV[2 * j:2 * j + 2, :], in_=fT[:])

            # ---------- V-MLP over 64 h-chunks, 4 chunks per pass ----------
            for m in range(0, NCHUNK, 4):
                psV1 = ps_tile("psV1")
                for c in range(4):
                    csl = slice((m + c) * NB, (m + c + 1) * NB)
                    nc.tensor.matmul(
                        psV1[64 * (c % 2):64 * (c % 2) + 64,
                             (c // 2) * NB:(c // 2) * NB + NB],
                        lhsT=C["v1e"][:, :], rhs=rhsV[:, csl],
                        start=True, stop=True, tile_position=(0, 64 * (c % 2)))
                E1 = wpool.tile([128, 2 * NB], F32, name="E1", tag="Ev")
                nc.scalar.activation(E1[:], psV1[:], AF.Exp, bias=C["bv1_e"][:, 0:1])
                R1 = wpool.tile([128, 2 * NB], F32, name="R1", tag="Rv")
                nc.vector.tensor_scalar(R1[:], psV1[:], C["bv1_r"][:, 0:1],
                                        1.0, ALU.add, ALU.max)
                O1 = wpool.tile([128, 2 * NB], F32R, name="O1", tag="Ov")
                nc.vector.tensor_tensor(O1[:], E1[:], R1[:], ALU.min)

                psV2 = ps_tile("psV2")
                for u in range(2):
                    bd_mm(C["v2s"], 0, O1[:, u * NB:(u + 1) * NB],
                          psV2[:, u * NB:(u + 1) * NB])
                E2 = wpool.tile([128, 2 * NB], F32, name="E2", tag="Ev")
                nc.scalar.activation(E2[:], psV2[:], AF.Exp, bias=C["bv2_e"][:, 0:1])
                R2 = wpool.tile([128, 2 * NB], F32, name="R2", tag="Rv")
                nc.vector.tensor_scalar(R2[:], psV2[:], C["bv2_r"][:, 0:1],
                                        1.0, ALU.add, ALU.max)
                O2 = wpool.tile([128, 2 * NB], F32R, name="O2", tag="Ov")
                nc.vector.tensor_tensor(O2[:], E2[:], R2[:], ALU.min)

                psV3 = ps_tile("psV3")
                for u in range(2):
                    bd_mm(C["v3s"], 0, O2[:, u * NB:(u + 1) * NB],
                          psV3[:, u * NB:(u + 1) * NB])
                E3 = wpool.tile([128, 2 * NB], F32, name="E3", tag="Ev")
                nc.scalar.activation(E3[:], psV3[:], AF.Exp, bias=C["bv3_e"][:, 0:1])
                R3 = wpool.tile([128, 2 * NB], F32, name="R3", tag="Rv")
                nc.vector.tensor_scalar(R3[:], psV3[:], C["bv3_r"][:, 0:1],
                                        1.0, ALU.add, ALU.max)
                O3 = wpool.tile([128, 2 * NB], F32R, name="O3", tag="Ov")
                nc.vector.tensor_tensor(O3[:], E3[:], R3[:], ALU.min)

                # V4: out [64, 2*NB]: rows 0-31 chunk even, 32-63 chunk odd
                psV4 = ps_tile("psV4", (64, 2 * NB))
                for u in range(2):
                    nc.tensor.matmul(
                        psV4[0:64, u * NB:(u + 1) * NB],
                        lhsT=C["v4s"][:, :], rhs=O3[:, u * NB:(u + 1) * NB],
                        start=True, stop=True, tile_position=(0, 0))
                O4 = wpool.tile([64, 2 * NB], F32R, name="O4", tag="O4")
                nc.scalar.activation(O4[:], psV4[:], AF.Identity,
                                     bias=C["bv4"][0:64, 0:1])
                # reverse collapse: chunk h = m + 2*pair + chalf
                # XR[(i%2)*64 + h, (i//2)*NB + b] with group pairing for xdot
                # O4 rows are parity-major (host permuted V4 columns):
                # row 32*chalf + 16*ip + i2  ->  group i = 2*i2 + ip
                for pair in range(2):
                    for chalf in range(2):
                        h = m + 2 * pair + chalf
                        for ip in range(2):
                            r0 = 32 * chalf + 16 * ip
                            src = O4[r0:r0 + 16, pair * NB:(pair + 1) * NB]
                            dst = XR[64 * ip + h:64 * ip + h + 1, :]
                            nc.sync.dma_start(out=dst, in_=src)

            # ---------- xdot ----------
            for j in range(NPAIR):
                psF = ps_tile("psF", (128, NB))
                bd_mm(C["wxd1"], j, XR[:, j * NB:(j + 1) * NB], psF[:])
                Ed = elu_evict(psF, C["bd1_e"][:, j:j + 1], C["bd1_r"][:, j:j + 1])
                psG = ps_tile("psG", (128, NB))
                bd_mm(C["wxd2"], j, Ed[:], psG[:])
                Of = wpool.tile([128, NB], F32, name="Of", tag="Of")
                nc.scalar.activation(Of[:], psG[:], AF.Identity,
                                     bias=C["b2d"][:, j:j + 1])
                nc.sync.dma_start(out=io["outT"][2 * j:2 * j + 2, :, tsl],
                                  in_=Of[:])


_NC_CACHE = None


def _get_nc():
    global _NC_CACHE
    if _NC_CACHE is None:
        _NC_CACHE = _build_nc()
    return _NC_CACHE


def _tf32(x):
    # round-to-nearest fp32 -> 19-bit (tf32-style) mantissa, matching the
    # PE's fp32r input precision so host data is pre-rounded
    xi = np.ascontiguousarray(x, np.float32).view(np.uint32)
    return ((xi + 0x1000) & 0xFFFFE000).view(np.float32)


def _host_prep(inputs):
    g = {k: np.asarray(v, np.float32) for k, v in inputs.items()}

    xWf = np.einsum("gab,gbc->gac", g["xenc_W2"], g["xext_W1"])
    bf_x = np.einsum("ga,gab->gb", g["xenc_b2"], g["xext_W1"]) + g["xext_b1"]
    zWf = np.einsum("gab,gbc->gac", g["zenc_W2"], g["zext_W1"])
    bf_z = np.einsum("ga,gab->gb", g["zenc_b2"], g["zext_W1"]) + g["zext_b1"]

    b2x_adj = g["xext_b2"] - g["xext_W2"].sum(axis=1)
    b2z_adj = g["zext_b2"] - g["zext_W2"].sum(axis=1)
    vb2_adj = g["vb2"] - g["V2"].sum(axis=0)
    vb3_adj = g["vb3"] - g["V3"].sum(axis=0)
    vb4_adj = g["vb4"] - g["V4"].sum(axis=0)
    b2d_adj = g["xdot_b2"] - g["xdot_W2"].sum(axis=1)

    V1 = g["V1"]
    V1p = V1[0:64] + V1[128:192]
    V1q = V1[64:128] - V1[128:192]

    def bd_stack(W):  # [32,64,64] -> [128, 16*128] block-diag pairs
        st = np.zeros((128, NPAIR * 128), np.float32)
        for j in range(NPAIR):
            st[0:64, j * 128:j * 128 + 64] = W[2 * j]
            st[64:128, j * 128 + 64:j * 128 + 128] = W[2 * j + 1]
        return st

    def pair_bias(b):  # [32,64] -> [128, 16]
        st = np.zeros((128, NPAIR), np.float32)
        for j in range(NPAIR):
            st[0:64, j] = b[2 * j]
            st[64:128, j] = b[2 * j + 1]
        return st

    def enc_mask(W1):  # [32,1,64] -> [128, 16*128] masked K=32 pair blocks
        st = np.zeros((128, NPAIR * 128), np.float32)
        for j in range(NPAIR):
            s = j % 4
            g0, g1 = 2 * j, 2 * j + 1
            st[32 * s + g0, j * 128:j * 128 + 64] = W1[g0, 0]
            st[32 * s + g1, j * 128 + 64:j * 128 + 128] = W1[g1, 0]
        return st

    dV2 = np.zeros((128, 128), np.float32)
    dV2[0:64, 0:64] = g["V2"]; dV2[64:128, 64:128] = g["V2"]
    dV3 = np.zeros((128, 128), np.float32)
    dV3[0:64, 0:64] = g["V3"]; dV3[64:128, 64:128] = g["V3"]
    # V4 column order parity-major: out row 16*(i%2) + i//2 holds group i
    v4perm = np.array([2 * (k % 16) + (k // 16) for k in range(32)])
    V4p = g["V4"][:, v4perm]
    dV4 = np.zeros((128, 64), np.float32)
    dV4[0:64, 0:32] = V4p; dV4[64:128, 32:64] = V4p

    const = {
        "wx1m": enc_mask(g["xenc_W1"]),
        "wz1m": enc_mask(g["zenc_W1"]),
        "wxf": bd_stack(xWf), "wzf": bd_stack(zWf),
        "wxe1": bd_stack(g["xext_W1"]), "wxe2": bd_stack(g["xext_W2"]),
        "wze2": bd_stack(g["zext_W2"]),
        "wxd1": bd_stack(g["xdot_W1"]), "wxd2": bd_stack(g["xdot_W2"]),
        "v1e": np.concatenate([V1p, V1q], axis=0),
        "v2s": dV2, "v3s": dV3, "v4s": dV4,
        "bxt": pair_bias(g["xenc_b1"]), "bzt": pair_bias(g["zenc_b1"]),
        "bfx_e": pair_bias(bf_x), "bfx_r": pair_bias(bf_x + 1.0),
        "bfz_e": pair_bias(bf_z), "bfz_r": pair_bias(bf_z + 1.0),
        "bx1_e": pair_bias(g["xext_b1"]), "bx1_r": pair_bias(g["xext_b1"] + 1.0),
        "b2x": pair_bias(b2x_adj), "b2z": pair_bias(b2z_adj),
        "bd1_e": pair_bias(g["xdot_b1"]), "bd1_r": pair_bias(g["xdot_b1"] + 1.0),
        "b2d": pair_bias(b2d_adj),
        "bv1_e": np.tile(g["vb1"], 2)[:, None],
        "bv1_r": np.tile(g["vb1"] + 1.0, 2)[:, None],
        "bv2_e": np.tile(vb2_adj, 2)[:, None],
        "bv2_r": np.tile(vb2_adj + 1.0, 2)[:, None],
        "bv3_e": np.tile(vb3_adj, 2)[:, None],
        "bv3_r": np.tile(vb3_adj + 1.0, 2)[:, None],
        "bv4": np.tile(vb4_adj[v4perm], 4)[:, None],
    }
    const = {k: np.ascontiguousarray(v) for k, v in const.items()}
    for k in ("wx1m", "wz1m", "wxf", "wzf", "wxe1", "wxe2", "wze2", "wxd1",
              "wxd2", "v2s", "v3s", "v4s"):
        const[k] = _tf32(const[k])

    def pack(vals, pk, width, np_dtype):
        arr = np.zeros((128, width), np_dtype)
        for nm, (p, off, w, rows) in CONST_LAYOUT.items():
            if p != pk:
                continue
            v = vals[nm].astype(np_dtype)
            assert v.shape == (rows, w), (nm, v.shape, rows, w)
            arr[0:rows, off:off + w] = v
        return arr

    in_maps = []
    for core in range(NCORES):
        sl = slice(core * BC, (core + 1) * BC)
        vals = dict(const)
        vals["x0r"] = _tf32(np.tile(g["x0"][sl, :, 0].T, (4, 1)))
        vals["z0r"] = _tf32(np.tile(g["z0"][sl, :, 0].T, (4, 1)))
        vals["ztr"] = _tf32(np.tile(g["zt"][sl, :, 0].T, (4, 1)))
        m = {
            "xhtT": _tf32(g["Xht"][sl].transpose(1, 2, 0)),
            "packR": pack(vals, "packR", PACKR_W, np.float32),
            "packF": pack(vals, "packF", PACKF_W, np.float32),
            "packB": pack(vals, "packB", PACKB_W, ml_dtypes.bfloat16),
        }
        in_maps.append(m)
    return in_maps


_LAST_RESULTS = None


def kernel(**inputs):
    global _LAST_RESULTS
    in_maps = _host_prep(inputs)
    nc = _get_nc()
    res = run_bass_kernel_spmd(nc, in_maps, core_ids=list(range(NCORES)))
    _LAST_RESULTS = res
    outs = [r["outT"].transpose(2, 0, 1) for r in res.results]
    return np.ascontiguousarray(np.concatenate(outs, axis=0)).astype(np.float32)


if __name__ == "__main__":
    print("smoke build only")
    _get_nc()
    print("built OK")

